# revision 1
# baseline (speedup 1.0000x reference)
"""Trainium2 Bass kernel for nn_Conv1d_NN_spatial (retrieval_knn).

Problem (per batch b, 8 batches -> 8 NeuronCores, data parallel):
  x [64, 4096] queries, y [64, 1024] keys
  dist2[n, m] = ||x_n||^2 + ||y_m||^2 - 2 x_n.y_m ; idx = 3 smallest per n
  out[oc, n] = relu(sum_k W_k @ x[:, idx[n, k]] + b)

Device algorithm (per core):
  key[n, m] = x_n.y_m - 0.5||y_m||^2  (maximize key <=> minimize dist; norm_x
  dropped - constant per row; sqrt dropped - monotone).
  The dot product is computed in 3-limb bf16 arithmetic (xh+xm+xl) so the key
  matches CPU-fp32 precision (~2e-6) at full bf16 PE speed: limb pairs
  (h,l)+(l,h), (m,m)+(h,m), (m,h)+(h,h) as three K=128 matmuls accumulated
  small->large into PSUM, plus a K=4 matmul adding -0.5||y||^2 (4 bf16 limbs).
  Top-3 per row via DVE max8/max_index. Conv reduced to a row gather of
  Zt[m] = [W_0^T x_m | W_1^T x_m | W_2^T x_m] + b/3 (built on device by a tiny
  fp32 matmul, stored m-major [1024, 192] in DRAM), 3 indirect-DMA gathers per
  128-row chunk (element_offset selects the k-th 64-col section), then a
  PSUM-accumulated PE transpose sums over k and yields [oc, n] directly; ACT
  applies ReLU.

Schedule: DVE (max8+max_index, 2.25us/chunk) is the saturated bottleneck.
  Input DMAs are split into pieces and issued in priority order across the SP
  and ACT HWDGE queues so the first chunk's operands land ~3us in; chunk 0/1
  front halves (key matmuls + copy + max) are emitted before the ZT build so
  DVE starts early; early-chunk gathers lag until ZT lands (deep i8 ring
  absorbs this); output is stored in 8 column groups as they complete.
"""
import sys

sys.path.insert(0, "/opt/trn_rl_repo")

import numpy as np
import ml_dtypes
from contextlib import ExitStack

import concourse.bass as bass
import concourse.tile as tile
from concourse import bacc, mybir
from concourse.bass import IndirectOffsetOnAxis
from concourse.bass_utils import run_bass_kernel_spmd

BF16 = ml_dtypes.bfloat16
B, C, N, M, K, OC = 8, 64, 4096, 1024, 3, 64
P = 128
NCHUNK = N // P  # 32
NPIECE = 8  # la/lb load pieces, 512 cols each
PCOLS = N // NPIECE


def _build(reps: int = 1):
    nc = bacc.Bacc("TRN2", target_bir_lowering=False, debug=False, num_devices=8)
    f32, bf16, u32 = mybir.dt.float32, mybir.dt.bfloat16, mybir.dt.uint32
    Relu = mybir.ActivationFunctionType.Relu

    LAd = nc.dram_tensor("la", [P, N], bf16, kind="ExternalInput").ap()
    LBd = nc.dram_tensor("lb", [P, N], bf16, kind="ExternalInput").ap()
    RAd = nc.dram_tensor("ra", [P, M], bf16, kind="ExternalInput").ap()
    RBd = nc.dram_tensor("rb", [P, M], bf16, kind="ExternalInput").ap()
    RCd = nc.dram_tensor("rc", [P, M], bf16, kind="ExternalInput").ap()
    RNd = nc.dram_tensor("rn", [4, M], bf16, kind="ExternalInput").ap()
    ONd = nc.dram_tensor("on", [4, P], bf16, kind="ExternalInput").ap()
    XCd = nc.dram_tensor("xc", [C + 1, M], f32, kind="ExternalInput").ap()
    WTd = nc.dram_tensor("wt", [C + 1, K * OC], f32, kind="ExternalInput").ap()
    IDd = nc.dram_tensor("idy", [P, P], f32, kind="ExternalInput").ap()
    OUTd = nc.dram_tensor("out", [OC, N], f32, kind="ExternalOutput").ap()

    with tile.TileContext(nc) as tc, ExitStack() as ctx:
        cn = ctx.enter_context(tc.tile_pool(name="cn", bufs=1))
        wk = ctx.enter_context(tc.tile_pool(name="wk", bufs=4))
        ix = ctx.enter_context(tc.tile_pool(name="ix", bufs=10))
        gk = ctx.enter_context(tc.tile_pool(name="gk", bufs=9))
        zw = ctx.enter_context(tc.tile_pool(name="zw", bufs=2))
        pk = ctx.enter_context(tc.tile_pool(name="pk", bufs=3, space="PSUM"))
        pt = ctx.enter_context(tc.tile_pool(name="pt", bufs=2, space="PSUM"))
        dr = ctx.enter_context(tc.tile_pool(name="dr", bufs=1, space="DRAM"))

        # ---- input loads: priority order, split across SP and ACT queues.
        # chunk-0 critical set first: la piece 0, RA on SP; RB/RC/RN/ON on ACT
        # (behind its fixed LoadActFuncSet); XC/WT for the ZT build + remaining
        # la/lb pieces go on SP so the ACT queue frees up for ks copies.
        LA, LB = [], []
        for t in range(NPIECE):
            la = cn.tile([P, PCOLS], bf16, tag=f"la{t}")
            LA.append(la)
            lb = cn.tile([P, PCOLS], bf16, tag=f"lb{t}")
            LB.append(lb)
        nc.sync.dma_start(LA[0][:], LAd[:, 0:PCOLS])
        RA = cn.tile([P, M], bf16)
        nc.sync.dma_start(RA[:], RAd[:])
        RB = cn.tile([P, M], bf16)
        nc.scalar.dma_start(RB[:], RBd[:])
        nc.sync.dma_start(LB[0][:], LBd[:, 0:PCOLS])
        RC = cn.tile([P, M], bf16)
        nc.scalar.dma_start(RC[:], RCd[:])
        RN = cn.tile([4, M], bf16)
        nc.sync.dma_start(RN[:], RNd[:])
        ON = cn.tile([4, P], bf16)
        nc.scalar.dma_start(ON[:], ONd[:])
        IDY = cn.tile([P, P], f32)
        nc.sync.dma_start(IDY[:], IDd[:])
        XC = cn.tile([C + 1, M], f32)
        nc.sync.dma_start(XC[:], XCd[:])
        WT = cn.tile([C + 1, K * OC], f32)
        nc.sync.dma_start(WT[:], WTd[:])
        for t in range(1, NPIECE):
            nc.sync.dma_start(LA[t][:], LAd[:, t * PCOLS:(t + 1) * PCOLS])
            nc.sync.dma_start(LB[t][:], LBd[:, t * PCOLS:(t + 1) * PCOLS])
        OUT_SB = cn.tile([OC, N], f32)
        # PE pstate warmup fodder: tiny memset tile, matmul'd before real work
        DM = cn.tile([4, 512], bf16, tag="dm")
        nc.gpsimd.memset(DM[:], 0)

        def body(_i=None):
            ZT = dr.tile([M, K * OC], f32)

            # warm the PE clock (p-state ramps only while continuously busy):
            # 3 dummy matmuls bridge the gap until chunk 0's operands land.
            for _ in range(3):
                dmy = pt.tile([P, 512], f32, tag="tr", space="PSUM")
                nc.tensor.matmul(dmy[:], DM[:, 0:P], DM[:], start=True, stop=True)

            def chunk_front(c):
                """key matmuls -> PSUM->SBUF copy -> max8/max_index"""
                la = LA[c // 4][:, (c % 4) * P:(c % 4 + 1) * P]
                lb = LB[c // 4][:, (c % 4) * P:(c % 4 + 1) * P]
                kp = pk.tile([P, M], f32, tag="kp", space="PSUM")
                for h in range(2):
                    hs = slice(h * 512, (h + 1) * 512)
                    nc.tensor.matmul(kp[:, hs], la, RA[:, hs], start=True, stop=False)
                for h in range(2):
                    hs = slice(h * 512, (h + 1) * 512)
                    nc.tensor.matmul(kp[:, hs], lb, RB[:, hs], start=False, stop=False)
                for h in range(2):
                    hs = slice(h * 512, (h + 1) * 512)
                    nc.tensor.matmul(kp[:, hs], lb, RC[:, hs], start=False, stop=False)
                for h in range(2):
                    hs = slice(h * 512, (h + 1) * 512)
                    nc.tensor.matmul(kp[:, hs], ON[:, :P], RN[:, hs], start=False,
                                     stop=True)
                m8 = wk.tile([P, 8], f32, tag="m8")
                i8 = ix.tile([P, 8], u32, tag="i8")
                if c == 0:
                    # chunk 0: scan PSUM directly - saves the copy latency on
                    # the critical path to the first Max (DVE pays +~130ns/op
                    # PSUM access, once)
                    nc.vector.max(m8[:], kp[:])
                    nc.vector.max_index(i8[:], m8[:], kp[:])
                else:
                    ks = wk.tile([P, M], f32, tag="ks")
                    nc.scalar.copy(ks[:], kp[:])
                    nc.vector.max(m8[:], ks[:])
                    nc.vector.max_index(i8[:], m8[:], ks[:])
                return i8

            def chunk_back(c, i8):
                """merged indirect gather -> PE transpose-accumulate -> ReLU

                One indirect DMA gathers the full 192-el ZT row of each of the
                3 neighbors (row r gets [row(i0) | row(i1) | row(i2)], 576 els)
                so only one descriptor-gen slice + one DMA semaphore edge per
                chunk; transpose k then reads the diagonal slice k*256..+64
                (neighbor k's section k)."""
                tr = pt.tile([OC, P], f32, tag="tr", space="PSUM")
                for k in range(K):
                    g = gk.tile([P, OC], f32, tag="g")
                    nc.gpsimd.indirect_dma_start(
                        out=g[:],
                        out_offset=None,
                        in_=ZT[:],
                        in_offset=IndirectOffsetOnAxis(ap=i8[:, k:k + 1], axis=0),
                        element_offset=k * OC,
                    )
                    nc.tensor.matmul(
                        tr[:], g[:], IDY[:], is_transpose=True,
                        start=(k == 0), stop=(k == K - 1),
                    )
                nc.scalar.activation(OUT_SB[:, c * P:(c + 1) * P], tr[:], Relu)
                # store finished 4-chunk output groups as they complete
                if c % 4 == 3:
                    g0 = c - 3
                    nc.sync.dma_start(
                        OUTd[:, g0 * P:(c + 1) * P], OUT_SB[:, g0 * P:(c + 1) * P]
                    )

            # ---- Zt table build: Zt[m, (k,oc)] = sum_c xc[c, m] wt[c, (k,oc)]
            # m-major [1024, 192]; row m = [W0^T x_m | W1^T x_m | W2^T x_m]+b/3
            def zt_step(t):
                zp = pt.tile([P, K * OC], f32, tag="tr", space="PSUM")
                nc.tensor.matmul(
                    zp[:], XC[:, t * P:(t + 1) * P], WT[:], start=True, stop=True
                )
                zs = zw.tile([P, K * OC], f32, tag="zs")
                nc.scalar.copy(zs[:], zp[:])
                nc.sync.dma_start(ZT[t * P:(t + 1) * P, :], zs[:])

            # chunk 0-5 fronts first so DVE saturates early, ZT build spread
            # 2 steps/chunk between them (all ZT PE matmuls precede the first
            # transpose matmul - gathers need the full table anyway); then
            # drain the pending backs and run the steady-state loop.
            NPRE = 6
            pend = []
            for c in range(NPRE):
                pend.append(chunk_front(c))
                if 2 <= c < NPRE:
                    zt_step(2 * (c - 2))
                    zt_step(2 * (c - 2) + 1)
            for c in range(NPRE):
                chunk_back(c, pend[c])
            for c in range(NPRE, NCHUNK):
                i8 = chunk_front(c)
                chunk_back(c, i8)

        if reps == 1:
            body()
        else:
            with tc.For_i(0, reps, 1) as i:
                body(i)

    nc.compile()
    return nc


_CACHE = {}


def _get_program(reps: int = 1):
    if reps not in _CACHE:
        _CACHE[reps] = _build(reps)
    return _CACHE[reps]


def _limbs(a):
    h = a.astype(BF16).astype(np.float32)
    m = (a - h).astype(BF16).astype(np.float32)
    l = (a - h - m).astype(BF16).astype(np.float32)
    return h, m, l


def prep_core_inputs(xb, yb, conv_w, conv_b):
    """Host-side prep for one batch: limb decomposition + aug tensors."""
    xh, xm, xl = _limbs(xb)
    yh, ym, yl = _limbs(yb)
    la = np.concatenate([xh, xl], 0).astype(BF16)
    lb = np.concatenate([xm, xh], 0).astype(BF16)
    ra = np.concatenate([yl, yh], 0).astype(BF16)
    rb = np.concatenate([ym, ym], 0).astype(BF16)
    rc = np.concatenate([yh, yh], 0).astype(BF16)
    nrm = -0.5 * (yb.astype(np.float64) ** 2).sum(0)
    rn = np.zeros((4, M), BF16)
    r = nrm
    for j in range(4):
        rn[j] = r.astype(BF16)
        r = r - rn[j].astype(np.float64)
    on = np.ones((4, P), BF16)
    xc = np.concatenate([xb[:, :M], np.ones((1, M), np.float32)], 0)
    wt = np.zeros((C + 1, K * OC), np.float32)
    for k in range(K):
        wt[:C, k * OC:(k + 1) * OC] = conv_w[:, :, k].T
        wt[C, k * OC:(k + 1) * OC] = conv_b / K
    idy = np.eye(P, dtype=np.float32)
    return {
        "la": la, "lb": lb, "ra": ra, "rb": rb, "rc": rc, "rn": rn,
        "on": on, "xc": xc, "wt": wt, "idy": idy,
    }


def _in_maps(x, y, conv_w, conv_b):
    return [prep_core_inputs(x[b], y[b], conv_w, conv_b) for b in range(B)]


def kernel(x, y, conv_w, conv_b):
    x = np.asarray(x, dtype=np.float32)
    y = np.asarray(y, dtype=np.float32)
    conv_w = np.asarray(conv_w, dtype=np.float32)
    conv_b = np.asarray(conv_b, dtype=np.float32)
    nc = _get_program(1)
    maps = _in_maps(x, y, conv_w, conv_b)
    res = run_bass_kernel_spmd(nc, maps, list(range(B)))
    return np.stack([res.results[b]["out"] for b in range(B)], 0)


def run_sim(x, y, conv_w, conv_b, core=0):
    """CoreSim single-core run for debugging."""
    from concourse.bass_interp import CoreSim

    nc = _get_program(1)
    maps = _in_maps(np.asarray(x, np.float32), np.asarray(y, np.float32),
                    np.asarray(conv_w, np.float32), np.asarray(conv_b, np.float32))
    sim = CoreSim(nc)
    for name, arr in maps[core].items():
        sim.tensor(name)[:] = arr
    sim.simulate(check_with_hw=False)
    return np.array(sim.tensor("out"))



# revision 26
# speedup vs baseline: 1.1193x; 1.1193x over previous
"""Trainium2 Bass kernel for nn_Conv1d_NN_spatial (retrieval_knn).

Problem (per batch b, 8 batches -> 8 NeuronCores, data parallel):
  x [64, 4096] queries, y [64, 1024] keys
  dist2[n, m] = ||x_n||^2 + ||y_m||^2 - 2 x_n.y_m ; idx = 3 smallest per n
  out[oc, n] = relu(sum_k W_k @ x[:, idx[n, k]] + b)

Device algorithm (per core):
  key[n, m] = x_n.y_m - 0.5||y_m||^2  (maximize key <=> minimize dist; norm_x
  dropped - constant per row; sqrt dropped - monotone).
  The dot product is computed in 3-limb bf16 arithmetic (xh+xm+xl) so the key
  matches CPU-fp32 precision at full bf16 PE speed. Three K=128 matmuls
  accumulated into PSUM:
    1: [xh(0:62),1,1, xl(0:62),1,1] . [yl(0:62),rn0,rn1, yh(0:62),rn2,rn3]
       = hl + lh (channels 0-61) + the 4 bf16 norm limbs rn of -0.5||y||^2
       (channels 62,63 of the ~1e-5-magnitude hl/lh terms are sacrificed for
       the norm rows; adds ~2e-5 key error, flips ~1 row in 32k)
    2: [xm, xh] . [ym, ym] = mm + hm
    3: [xm, xh] . [yh, yh] = mh + hh
  Top-3 per row via DVE max8/max_index (the saturated bottleneck engine,
  2 x 1024-el fp32 passes = ~2.25us/chunk x 32 chunks). Conv reduced to a row
  gather of Zt[m] = [W_0^T x_m | W_1^T x_m | W_2^T x_m] + b/3 (built on device
  by tiny fp32 matmuls, stored m-major [1024, 192] in DRAM). ONE merged
  indirect DMA per 128-row chunk gathers all 3 neighbors' full 192-el rows
  (offset ap i8[:, 0:3], 384 descriptors -> one 994ns SWDGE fixed cost instead
  of three); transpose-matmul k reads the diagonal slice k*192+k*64..+64 and
  PSUM-accumulates over k, yielding [oc, n] directly; ACT applies ReLU.

Schedule: DVE-limited. Inputs are batched into 6 dma_starts (HWDGE fixed cost
  is 625ns each): rabc half 0, lab chunks 0-3, rabc half 1, then the rest.
  lab interleaves [la_c | lb_c] per 128-col chunk so one contiguous DMA covers
  chunks 4-31. The ZT build runs first on PE (also warming the p-state after
  a few dummy matmuls); fronts 0/1 scan PSUM directly (skip the ACT copy) to
  cut fill latency; backs lag fronts by 2 chunks so PE's in-order queue never
  head-of-line blocks key matmuls behind a gather semaphore.
"""
import sys

sys.path.insert(0, "/opt/trn_rl_repo")

import numpy as np
import ml_dtypes
from contextlib import ExitStack

import concourse.bass as bass
import concourse.tile as tile
from concourse import bacc, mybir
from concourse.bass import IndirectOffsetOnAxis
from concourse.bass_utils import run_bass_kernel_spmd

BF16 = ml_dtypes.bfloat16
B, C, N, M, K, OC = 8, 64, 4096, 1024, 3, 64
P = 128
NCHUNK = N // P  # 32
ZROW = K * OC  # 192

# schedule knobs (tuned via TimelineSim sweep). Emission lags per stage keep
# every engine's in-order queue free of head-of-line semaphore waits: each
# stage's input sem has already fired by the time its iteration comes up.
SLAG = 1    # wrap store(c) at iter c+SLAG (SP queue)
LLAG = 2    # wrap load(c) at iter c+LLAG (SP queue)
GLAG = 4    # dma_gather(c) at iter c+GLAG (Pool queue)
TLAG = 6    # transpose(c) at iter c+TLAG (PE queue)
RLAG = 8    # relu(c) at iter c+RLAG (ACT queue)
OLAG = 10   # 4-chunk output store at iter c+OLAG (ACT queue)
NDUM = 8    # PE warmup dummy matmuls
ZPOS = 4    # zt build emitted after front(ZPOS-1)
NDIRECT = 2  # first chunks whose max/max_index scan PSUM directly


def _build(reps: int = 1):
    nc = bacc.Bacc("TRN2", target_bir_lowering=False, debug=False, num_devices=8)
    f32, bf16, u32 = mybir.dt.float32, mybir.dt.bfloat16, mybir.dt.uint32
    i16 = mybir.dt.int16
    Relu = mybir.ActivationFunctionType.Relu

    LABd = nc.dram_tensor("lab", [P, 2 * N], bf16, kind="ExternalInput").ap()
    RABCd = nc.dram_tensor("rabc", [P, 3 * M], bf16, kind="ExternalInput").ap()
    XWd = nc.dram_tensor("xw", [C + 1, M + ZROW], f32, kind="ExternalInput").ap()
    IDd = nc.dram_tensor("idy", [P, P], f32, kind="ExternalInput").ap()
    OUTd = nc.dram_tensor("out", [OC, N], f32, kind="ExternalOutput").ap()

    with tile.TileContext(nc) as tc, ExitStack() as ctx:
        cn = ctx.enter_context(tc.tile_pool(name="cn", bufs=1))
        wk = ctx.enter_context(tc.tile_pool(name="wk", bufs=4))
        ix = ctx.enter_context(tc.tile_pool(name="ix", bufs=14))
        gk = ctx.enter_context(tc.tile_pool(name="gk", bufs=8))
        wq = ctx.enter_context(tc.tile_pool(name="wq", bufs=8))
        zw = ctx.enter_context(tc.tile_pool(name="zw", bufs=2))
        pk = ctx.enter_context(tc.tile_pool(name="pk", bufs=3, space="PSUM"))
        pt = ctx.enter_context(tc.tile_pool(name="pt", bufs=2, space="PSUM"))
        dr = ctx.enter_context(tc.tile_pool(name="dr", bufs=1, space="DRAM"))
        dw = ctx.enter_context(tc.tile_pool(name="dw", bufs=8, space="DRAM"))

        # ---- input loads: few big DMAs in priority order (HWDGE serializes
        # dma_starts at ~625ns each and the DMA engines drain the queue
        # serially at ~360GB/s). Chunk 0/1's operands land first.
        RABC0 = cn.tile([P, 3 * 512], bf16, tag="rabc0")
        nc.sync.dma_start(RABC0[:], RABCd[:, 0 : 3 * 512])
        LAB01 = cn.tile([P, 512], bf16, tag="lab01")
        nc.scalar.dma_start(LAB01[:], LABd[:, 0:512])
        RABC1 = cn.tile([P, 3 * 512], bf16, tag="rabc1")
        nc.sync.dma_start(RABC1[:], RABCd[:, 3 * 512 : 3 * 1024])
        RABC = (RABC0, RABC1)
        LAB23 = cn.tile([P, 512], bf16, tag="lab23")
        nc.scalar.dma_start(LAB23[:], LABd[:, 512:1024])
        XW = cn.tile([C + 1, M + ZROW], f32)
        nc.sync.dma_start(XW[:], XWd[:])
        IDY = cn.tile([P, P], f32)
        nc.scalar.dma_start(IDY[:], IDd[:])
        LABR = cn.tile([P, 2 * N - 1024], bf16, tag="labr")
        nc.sync.dma_start(LABR[:], LABd[:, 1024 : 2 * N])
        OUT_SB = cn.tile([OC, N], f32)
        # PE pstate warmup fodder: tiny zeroed tile, matmul'd before real work
        DM = cn.tile([4, 512], bf16, tag="dm")
        nc.vector.memset(DM[:], 0)
        NIDX_REG = nc.gpsimd.to_reg(K * P)

        def body(_i=None):
            ZT = dr.tile([M, ZROW], f32)

            # warm the PE clock (p-state ramps only while continuously busy):
            # dummy matmuls bridge the gap until chunk 0's operands land.
            for _ in range(NDUM):
                dmy = pt.tile([P, 512], f32, tag="tr", space="PSUM")
                nc.tensor.matmul(dmy[:], DM[:, 0:P], DM[:], start=True, stop=True)

            def zt_build():
                # Zt[m, (k,oc)] = sum_c xc[c,m] wt[c,(k,oc)], m-major [1024,192]
                # stores go on the ACT HWDGE queue: SP's queue is budgeted for
                # the per-chunk wrap store+load pair
                for t in range(M // P):
                    zp = pt.tile([P, ZROW], f32, tag="tr", space="PSUM")
                    nc.tensor.matmul(
                        zp[:], XW[:, t * P : (t + 1) * P], XW[:, M : M + ZROW],
                        start=True, stop=True,
                    )
                    zs = zw.tile([P, ZROW], f32, tag="zs")
                    nc.scalar.copy(zs[:], zp[:])
                    nc.scalar.dma_start(ZT[t * P : (t + 1) * P, :], zs[:])

            def chunk_front(c):
                """key matmuls -> PSUM->SBUF copy -> max8/max_index"""
                if c < 2:
                    la = LAB01[:, c * 256 : c * 256 + P]
                    lb = LAB01[:, c * 256 + P : (c + 1) * 256]
                elif c < 4:
                    la = LAB23[:, (c - 2) * 256 : (c - 2) * 256 + P]
                    lb = LAB23[:, (c - 2) * 256 + P : (c - 1) * 256]
                else:
                    base = (c - 4) * 256
                    la = LABR[:, base : base + P]
                    lb = LABR[:, base + P : base + 256]
                kp = pk.tile([P, M], f32, tag="kp", space="PSUM")
                for h in range(2):
                    hs = slice(h * 512, (h + 1) * 512)
                    nc.tensor.matmul(kp[:, hs], la, RABC[h][:, 0:512],
                                     start=True, stop=False)
                for h in range(2):
                    hs = slice(h * 512, (h + 1) * 512)
                    nc.tensor.matmul(kp[:, hs], lb, RABC[h][:, 512:1024],
                                     start=False, stop=False)
                for h in range(2):
                    hs = slice(h * 512, (h + 1) * 512)
                    nc.tensor.matmul(kp[:, hs], lb, RABC[h][:, 1024:1536],
                                     start=False, stop=True)
                m8 = wk.tile([P, 8], f32, tag="m8")
                i8 = ix.tile([P, 8], u32, tag="i8")
                if c < NDIRECT:
                    # early chunks: scan PSUM directly - saves the copy latency
                    # on the critical path to the first Max (DVE pays +~65ns/op
                    # PSUM access, twice)
                    nc.vector.max(m8[:], kp[:])
                    nc.vector.max_index(i8[:], m8[:], kp[:])
                else:
                    ks = wk.tile([P, M], f32, tag="ks")
                    nc.scalar.copy(ks[:], kp[:])
                    nc.vector.max(m8[:], ks[:])
                    nc.vector.max_index(i8[:], m8[:], ks[:])
                return i8

            # The merged gather needs its 384 indices 16-partition-wrapped and
            # replicated (wrapped[q, 8j+a] = i8[16a+q, j], same in every
            # 16-partition group). That cross-partition wrap is a pure-AP DRAM
            # round-trip costing no compute engine time: store the u32 low
            # halves through a wrap-ordered DRAM AP, load back through a
            # stride-0-broadcast AP.
            def wrap_store(c, i8):
                wd = dw.tile([16, K * 8], i16, tag="wd")
                nc.sync.dma_start(
                    wd[:].rearrange("q (j a) -> a q j", j=K, a=8),
                    i8[:].bitcast(i16)[:, 0 : 2 * K : 2],
                )
                return wd

            def wrap_load(c, wd):
                wr = wq.tile([P, K * 8], i16, tag="wr")
                nc.sync.dma_start(
                    wr[:], wd[:].unsqueeze(0).broadcast_to([8, 16, K * 8])
                )
                return wr

            def chunk_gather(c, wr):
                """ONE merged dma_gather per chunk: the full 192-el ZT row of
                each of the 3 neighbors lands in g (row r = [row(i0)|row(i1)|
                row(i2)], 576 els) - one 994ns SWDGE descriptor-gen pass
                instead of three."""
                g = gk.tile([P, K * ZROW], f32, tag="g")
                nc.gpsimd.dma_gather(
                    out_ap=g[:].rearrange("p (j w) -> p j w", j=K),
                    in_ap=ZT[:],
                    idxs_ap=wr[:],
                    num_idxs=K * P,
                    num_idxs_reg=NIDX_REG,
                    elem_size=ZROW,
                )
                return g

            def chunk_transpose(c, g):
                """PE transpose-accumulate: transpose k reads the diagonal
                slice k*192 + k*64 .. +64 (neighbor k's section k)."""
                tr = pt.tile([OC, P], f32, tag="tr", space="PSUM")
                for k in range(K):
                    sl = slice(k * ZROW + k * OC, k * ZROW + (k + 1) * OC)
                    nc.tensor.matmul(
                        tr[:], g[:, sl], IDY[:], is_transpose=True,
                        start=(k == 0), stop=(k == K - 1),
                    )
                return tr

            def chunk_relu(c, tr):
                nc.scalar.activation(OUT_SB[:, c * P:(c + 1) * P], tr[:], Relu)

            def chunk_store(c):
                # store finished 4-chunk output groups, lagged so the relu
                # sems have fired by dispatch time (ACT HWDGE queue - SP's is
                # budgeted for the wrap store+load pair)
                if c % 4 == 3:
                    g0 = c - 3
                    nc.scalar.dma_start(
                        OUTd[:, g0 * P:(c + 1) * P], OUT_SB[:, g0 * P:(c + 1) * P]
                    )

            i8s, wds, wrs, gs, trs = {}, {}, {}, {}, {}

            def step(c):
                if c < NCHUNK:
                    i8s[c] = chunk_front(c)
                if ZPOS == c:
                    zt_build()
                if 0 <= c - SLAG < NCHUNK:
                    j = c - SLAG
                    wds[j] = wrap_store(j, i8s.pop(j))
                if 0 <= c - LLAG < NCHUNK:
                    j = c - LLAG
                    wrs[j] = wrap_load(j, wds.pop(j))
                if 0 <= c - GLAG < NCHUNK:
                    j = c - GLAG
                    gs[j] = chunk_gather(j, wrs.pop(j))
                if 0 <= c - TLAG < NCHUNK:
                    j = c - TLAG
                    trs[j] = chunk_transpose(j, gs.pop(j))
                if 0 <= c - RLAG < NCHUNK:
                    j = c - RLAG
                    chunk_relu(j, trs.pop(j))
                if 0 <= c - OLAG < NCHUNK:
                    chunk_store(c - OLAG)

            for c in range(NCHUNK + OLAG):
                step(c)

        if reps == 1:
            body()
        else:
            with tc.For_i(0, reps, 1) as i:
                body(i)

    nc.compile()
    return nc


_CACHE = {}


def _get_program(reps: int = 1):
    if reps not in _CACHE:
        _CACHE[reps] = _build(reps)
    return _CACHE[reps]


def _limbs(a):
    h = a.astype(BF16).astype(np.float32)
    m = (a - h).astype(BF16).astype(np.float32)
    l = (a - h - m).astype(BF16).astype(np.float32)
    return h, m, l


def prep_core_inputs(xb, yb, conv_w, conv_b):
    """Host-side prep for one batch: limb decomposition + aug tensors."""
    xh, xm, xl = _limbs(xb)
    yh, ym, yl = _limbs(yb)
    ones2 = np.ones((2, N), np.float32)
    la = np.concatenate([xh[: C - 2], ones2, xl[: C - 2], ones2], 0)
    lb = np.concatenate([xm, xh], 0)
    # lab interleaves [la_c | lb_c] per 128-col chunk -> one contiguous DMA
    # covers any chunk range
    lab = np.empty((P, 2 * N), np.float32)
    lab3 = lab.reshape(P, NCHUNK, 2 * P)
    lab3[:, :, :P] = la.reshape(P, NCHUNK, P)
    lab3[:, :, P:] = lb.reshape(P, NCHUNK, P)
    # 4 bf16 limbs of -0.5||y||^2 go in the rows sacrificed from hl/lh
    nrm = -0.5 * (yb.astype(np.float64) ** 2).sum(0)
    rn = np.zeros((4, M), np.float32)
    r = nrm
    for j in range(4):
        rn[j] = r.astype(BF16).astype(np.float32)
        r = r - rn[j].astype(np.float64)
    ra = np.concatenate([yl[: C - 2], rn[0:2], yh[: C - 2], rn[2:4]], 0)
    rb = np.concatenate([ym, ym], 0)
    rc = np.concatenate([yh, yh], 0)
    # rabc packs the two 512-col halves: [RA_h | RB_h | RC_h] for h = 0, 1
    rabc = np.empty((P, 3 * M), np.float32)
    for h in range(2):
        o = h * 3 * 512
        s = slice(h * 512, (h + 1) * 512)
        rabc[:, o : o + 512] = ra[:, s]
        rabc[:, o + 512 : o + 1024] = rb[:, s]
        rabc[:, o + 1024 : o + 1536] = rc[:, s]
    xw = np.zeros((C + 1, M + ZROW), np.float32)
    xw[:C, :M] = xb[:, :M]
    xw[C, :M] = 1.0
    for k in range(K):
        xw[:C, M + k * OC : M + (k + 1) * OC] = conv_w[:, :, k].T
        xw[C, M + k * OC : M + (k + 1) * OC] = conv_b / K
    idy = np.eye(P, dtype=np.float32)
    return {
        "lab": lab.astype(BF16), "rabc": rabc.astype(BF16),
        "xw": xw, "idy": idy,
    }


def _in_maps(x, y, conv_w, conv_b):
    return [prep_core_inputs(x[b], y[b], conv_w, conv_b) for b in range(B)]


def kernel(x, y, conv_w, conv_b):
    x = np.asarray(x, dtype=np.float32)
    y = np.asarray(y, dtype=np.float32)
    conv_w = np.asarray(conv_w, dtype=np.float32)
    conv_b = np.asarray(conv_b, dtype=np.float32)
    nc = _get_program(1)
    maps = _in_maps(x, y, conv_w, conv_b)
    res = run_bass_kernel_spmd(nc, maps, list(range(B)))
    return np.stack([res.results[b]["out"] for b in range(B)], 0)


def run_sim(x, y, conv_w, conv_b, core=0):
    """CoreSim single-core run for debugging."""
    from concourse.bass_interp import CoreSim

    nc = _get_program(1)
    maps = _in_maps(np.asarray(x, np.float32), np.asarray(y, np.float32),
                    np.asarray(conv_w, np.float32), np.asarray(conv_b, np.float32))
    sim = CoreSim(nc)
    for name, arr in maps[core].items():
        sim.tensor(name)[:] = arr
    sim.simulate(check_with_hw=False)
    return np.array(sim.tensor("out"))


# revision 35
# speedup vs baseline: 2.0383x; 1.8211x over previous
"""Trainium2 Bass kernel for nn_Conv1d_NN_spatial (retrieval_knn).

Problem (per batch b, 8 batches -> 8 NeuronCores, data parallel):
  x [64, 4096] queries, y [64, 1024] keys
  dist2[n, m] = ||x_n||^2 + ||y_m||^2 - 2 x_n.y_m ; idx = 3 smallest per n
  out[oc, n] = relu(sum_k W_k @ x[:, idx[n, k]] + b)

Device algorithm (per core):
  key[n, m] = x_n.y_m - 0.5||y_m||^2  (maximize key <=> minimize dist; norm_x
  dropped - constant per row; sqrt dropped - monotone).
  The dot product is computed in 3-limb bf16 arithmetic (xh+xm+xl) so the key
  matches CPU-fp32 precision at full bf16 PE speed. Three K=128 matmuls
  accumulated into PSUM:
    1: [xh(0:62),1,1, xl(0:62),1,1] . [yl(0:62),rn0,rn1, yh(0:62),rn2,rn3]
       = hl + lh (channels 0-61) + the 4 bf16 norm limbs rn of -0.5||y||^2
       (channels 62,63 of the ~1e-5-magnitude hl/lh terms are sacrificed for
       the norm rows; adds ~2e-5 key error, flips ~1 row in 32k)
    2: [xm, xh] . [ym, ym] = mm + hm
    3: [xm, xh] . [yh, yh] = mh + hh
  Top-3 per row via DVE max8/max_index (the saturated bottleneck engine,
  2 x 1024-el fp32 passes = ~2.25us/chunk x 32 chunks = 72us; everything else
  is scheduled around keeping DVE 100% busy). Conv reduced to a row gather of
  Zt[m] = [W_0^T x_m | W_1^T x_m | W_2^T x_m] + b/3 (built on device by tiny
  fp32 matmuls, stored m-major [1024, 192] in DRAM).

Gather: ONE gpsimd.dma_gather per 128-row chunk fetches all 3 neighbors' full
  192-el rows (384 descriptors -> one 994ns SWDGE descriptor-gen pass instead
  of three; out[p, j, :] = ZT[i8[p, j]]). dma_gather's descriptor generator
  reads its 384 indices from a 16-partition-wrapped, replicated int16 tile
  (wrapped[q, 8j+a] = i8[16a+q, j] - HW-probed; the multi-index form of
  indirect_dma_start reads only index 0 per partition on real HW). That
  cross-partition wrap is produced by a pure-AP DRAM round-trip costing no
  compute-engine time: store the u32 low halves through a wrap-ordered DRAM
  AP, load back through a stride-0-broadcast AP. Transpose-matmul k then
  reads the diagonal slice k*192+k*64..+64 of the gathered rows and
  PSUM-accumulates over k, yielding [oc, n] directly; ACT applies ReLU.

Schedule: DVE-limited. Inputs are batched into 7 dma_starts (HWDGE fixed cost
  is ~625ns each): rabc half 0, lab chunks 0-1, rabc half 1, lab 2-3, xw,
  idy, lab rest. lab interleaves [la_c | lb_c] per 128-col chunk so one
  contiguous DMA covers chunks 4-31. PE warms its p-state on dummy matmuls;
  fronts 0/1 scan PSUM directly (skip the ACT copy) to cut fill latency.
  Each back-half stage (wrap store SP / wrap load SP / gather Pool /
  transpose PE / relu ACT / out-store ACT) is emitted with its own per-stage
  lag so no engine's in-order queue ever head-of-line blocks on a semaphore
  that has not fired yet (lags swept via TimelineSim).
"""
import sys

sys.path.insert(0, "/opt/trn_rl_repo")

import numpy as np
import ml_dtypes
from contextlib import ExitStack

import concourse.bass as bass
import concourse.tile as tile
from concourse import bacc, mybir
from concourse.bass import IndirectOffsetOnAxis
from concourse.bass_utils import run_bass_kernel_spmd

BF16 = ml_dtypes.bfloat16
B, C, N, M, K, OC = 8, 64, 4096, 1024, 3, 64
P = 128
NCHUNK = N // P  # 32
ZROW = K * OC  # 192

# schedule knobs (tuned via TimelineSim sweep). Emission lags per stage keep
# every engine's in-order queue free of head-of-line semaphore waits: each
# stage's input sem has already fired by the time its iteration comes up.
SLAG = 1    # wrap store(c) at iter c+SLAG (SP queue)
LLAG = 2    # wrap load(c) at iter c+LLAG (SP queue)
GLAG = 4    # dma_gather(c) at iter c+GLAG (Pool queue)
TLAG = 6    # transpose(c) at iter c+TLAG (PE queue)
RLAG = 8    # relu(c) at iter c+RLAG (ACT queue)
OLAG = 10   # 4-chunk output store at iter c+OLAG (ACT queue)
NDUM = 10   # PE warmup dummy matmuls
ZPOS = 4    # zt build emitted after front(ZPOS-1)
NDIRECT = 2  # first chunks whose max/max_index scan PSUM directly


def _build(reps: int = 1):
    nc = bacc.Bacc("TRN2", target_bir_lowering=False, debug=False, num_devices=8)
    f32, bf16, u32 = mybir.dt.float32, mybir.dt.bfloat16, mybir.dt.uint32
    i16 = mybir.dt.int16
    Relu = mybir.ActivationFunctionType.Relu

    LABd = nc.dram_tensor("lab", [P, 2 * N], bf16, kind="ExternalInput").ap()
    RABCd = nc.dram_tensor("rabc", [P, 3 * M], bf16, kind="ExternalInput").ap()
    XWd = nc.dram_tensor("xw", [C + 1, M + ZROW], f32, kind="ExternalInput").ap()
    IDd = nc.dram_tensor("idy", [P, P], f32, kind="ExternalInput").ap()
    OUTd = nc.dram_tensor("out", [OC, N], f32, kind="ExternalOutput").ap()

    with tile.TileContext(nc) as tc, ExitStack() as ctx:
        cn = ctx.enter_context(tc.tile_pool(name="cn", bufs=1))
        wk = ctx.enter_context(tc.tile_pool(name="wk", bufs=4))
        ix = ctx.enter_context(tc.tile_pool(name="ix", bufs=14))
        gk = ctx.enter_context(tc.tile_pool(name="gk", bufs=8))
        wq = ctx.enter_context(tc.tile_pool(name="wq", bufs=8))
        zw = ctx.enter_context(tc.tile_pool(name="zw", bufs=2))
        pk = ctx.enter_context(tc.tile_pool(name="pk", bufs=3, space="PSUM"))
        pt = ctx.enter_context(tc.tile_pool(name="pt", bufs=2, space="PSUM"))
        dr = ctx.enter_context(tc.tile_pool(name="dr", bufs=1, space="DRAM"))
        dw = ctx.enter_context(tc.tile_pool(name="dw", bufs=8, space="DRAM"))

        # ---- input loads: batched DMAs in priority order (HWDGE serializes
        # dma_starts at ~625ns each and the DMA engines drain the queue
        # serially at ~360GB/s). Chunk 0's six operand pieces land first, in
        # the order its matmuls consume them.
        RABC0 = cn.tile([P, 3 * 512], bf16, tag="rabc0")
        nc.sync.dma_start(RABC0[:], RABCd[:, 0 : 3 * 512])
        LAB01 = cn.tile([P, 512], bf16, tag="lab01")
        nc.scalar.dma_start(LAB01[:], LABd[:, 0:512])
        RABC1 = cn.tile([P, 3 * 512], bf16, tag="rabc1")
        nc.sync.dma_start(RABC1[:], RABCd[:, 3 * 512 : 3 * 1024])
        RABC = [[RABC0[:, r * 512:(r + 1) * 512] for r in range(3)],
                [RABC1[:, r * 512:(r + 1) * 512] for r in range(3)]]
        LAB23 = cn.tile([P, 512], bf16, tag="lab23")
        nc.scalar.dma_start(LAB23[:], LABd[:, 512:1024])
        XW = cn.tile([C + 1, M + ZROW], f32)
        nc.sync.dma_start(XW[:], XWd[:])
        IDY = cn.tile([P, P], f32)
        nc.scalar.dma_start(IDY[:], IDd[:])
        LABR = cn.tile([P, 2 * N - 1024], bf16, tag="labr")
        nc.sync.dma_start(LABR[:], LABd[:, 1024 : 2 * N])
        OUT_SB = cn.tile([OC, N], f32)
        # PE pstate warmup fodder: tiny zeroed tile, matmul'd before real work
        DM = cn.tile([4, 512], bf16, tag="dm")
        nc.vector.memset(DM[:], 0)
        NIDX_REG = nc.gpsimd.to_reg(K * P)

        def body(_i=None):
            ZT = dr.tile([M, ZROW], f32)

            # warm the PE clock (p-state ramps only while continuously busy):
            # dummy matmuls bridge the gap until chunk 0's operands land.
            for _ in range(NDUM):
                dmy = pt.tile([P, 512], f32, tag="tr", space="PSUM")
                nc.tensor.matmul(dmy[:], DM[:, 0:P], DM[:], start=True, stop=True)

            def zt_build():
                # Zt[m, (k,oc)] = sum_c xc[c,m] wt[c,(k,oc)], m-major [1024,192]
                # stores go on the ACT HWDGE queue: SP's queue is budgeted for
                # the per-chunk wrap store+load pair
                for t in range(M // P):
                    zp = pt.tile([P, ZROW], f32, tag="tr", space="PSUM")
                    nc.tensor.matmul(
                        zp[:], XW[:, t * P : (t + 1) * P], XW[:, M : M + ZROW],
                        start=True, stop=True,
                    )
                    zs = zw.tile([P, ZROW], f32, tag="zs")
                    nc.scalar.copy(zs[:], zp[:])
                    nc.scalar.dma_start(ZT[t * P : (t + 1) * P, :], zs[:])

            def chunk_front(c):
                """key matmuls -> PSUM->SBUF copy -> max8/max_index"""
                if c < 2:
                    la = LAB01[:, c * 256 : c * 256 + P]
                    lb = LAB01[:, c * 256 + P : (c + 1) * 256]
                elif c < 4:
                    la = LAB23[:, (c - 2) * 256 : (c - 2) * 256 + P]
                    lb = LAB23[:, (c - 2) * 256 + P : (c - 1) * 256]
                else:
                    base = (c - 4) * 256
                    la = LABR[:, base : base + P]
                    lb = LABR[:, base + P : base + 256]
                kp = pk.tile([P, M], f32, tag="kp", space="PSUM")
                for h in range(2):
                    hs = slice(h * 512, (h + 1) * 512)
                    nc.tensor.matmul(kp[:, hs], la, RABC[h][0],
                                     start=True, stop=False)
                for h in range(2):
                    hs = slice(h * 512, (h + 1) * 512)
                    nc.tensor.matmul(kp[:, hs], lb, RABC[h][1],
                                     start=False, stop=False)
                for h in range(2):
                    hs = slice(h * 512, (h + 1) * 512)
                    nc.tensor.matmul(kp[:, hs], lb, RABC[h][2],
                                     start=False, stop=True)
                m8 = wk.tile([P, 8], f32, tag="m8")
                i8 = ix.tile([P, 8], u32, tag="i8")
                if c < NDIRECT:
                    # early chunks: scan PSUM directly - saves the copy latency
                    # on the critical path to the first Max (DVE pays +~65ns/op
                    # PSUM access, twice)
                    nc.vector.max(m8[:], kp[:])
                    nc.vector.max_index(i8[:], m8[:], kp[:])
                else:
                    ks = wk.tile([P, M], f32, tag="ks")
                    nc.scalar.copy(ks[:], kp[:])
                    nc.vector.max(m8[:], ks[:])
                    nc.vector.max_index(i8[:], m8[:], ks[:])
                return i8

            # The merged gather needs its 384 indices 16-partition-wrapped and
            # replicated (wrapped[q, 8j+a] = i8[16a+q, j], same in every
            # 16-partition group). That cross-partition wrap is a pure-AP DRAM
            # round-trip costing no compute engine time: store the u32 low
            # halves through a wrap-ordered DRAM AP, load back through a
            # stride-0-broadcast AP.
            def wrap_store(c, i8):
                wd = dw.tile([16, K * 8], i16, tag="wd")
                nc.sync.dma_start(
                    wd[:].rearrange("q (j a) -> a q j", j=K, a=8),
                    i8[:].bitcast(i16)[:, 0 : 2 * K : 2],
                )
                return wd

            def wrap_load(c, wd):
                wr = wq.tile([P, K * 8], i16, tag="wr")
                nc.sync.dma_start(
                    wr[:], wd[:].unsqueeze(0).broadcast_to([8, 16, K * 8])
                )
                return wr

            def chunk_gather(c, wr):
                """ONE merged dma_gather per chunk: the full 192-el ZT row of
                each of the 3 neighbors lands in g (row r = [row(i0)|row(i1)|
                row(i2)], 576 els) - one 994ns SWDGE descriptor-gen pass
                instead of three."""
                g = gk.tile([P, K * ZROW], f32, tag="g")
                nc.gpsimd.dma_gather(
                    out_ap=g[:].rearrange("p (j w) -> p j w", j=K),
                    in_ap=ZT[:],
                    idxs_ap=wr[:],
                    num_idxs=K * P,
                    num_idxs_reg=NIDX_REG,
                    elem_size=ZROW,
                )
                return g

            def chunk_transpose(c, g):
                """PE transpose-accumulate: transpose k reads the diagonal
                slice k*192 + k*64 .. +64 (neighbor k's section k)."""
                tr = pt.tile([OC, P], f32, tag="tr", space="PSUM")
                for k in range(K):
                    sl = slice(k * ZROW + k * OC, k * ZROW + (k + 1) * OC)
                    nc.tensor.matmul(
                        tr[:], g[:, sl], IDY[:], is_transpose=True,
                        start=(k == 0), stop=(k == K - 1),
                    )
                return tr

            def chunk_relu(c, tr):
                nc.scalar.activation(OUT_SB[:, c * P:(c + 1) * P], tr[:], Relu)

            def chunk_store(c):
                # store finished 4-chunk output groups, lagged so the relu
                # sems have fired by dispatch time (ACT HWDGE queue - SP's is
                # budgeted for the wrap store+load pair)
                if c % 4 == 3:
                    g0 = c - 3
                    nc.scalar.dma_start(
                        OUTd[:, g0 * P:(c + 1) * P], OUT_SB[:, g0 * P:(c + 1) * P]
                    )

            i8s, wds, wrs, gs, trs = {}, {}, {}, {}, {}

            def step(c):
                if c < NCHUNK:
                    i8s[c] = chunk_front(c)
                if ZPOS == c:
                    zt_build()
                if 0 <= c - SLAG < NCHUNK:
                    j = c - SLAG
                    wds[j] = wrap_store(j, i8s.pop(j))
                if 0 <= c - LLAG < NCHUNK:
                    j = c - LLAG
                    wrs[j] = wrap_load(j, wds.pop(j))
                if 0 <= c - GLAG < NCHUNK:
                    j = c - GLAG
                    gs[j] = chunk_gather(j, wrs.pop(j))
                if 0 <= c - TLAG < NCHUNK:
                    j = c - TLAG
                    trs[j] = chunk_transpose(j, gs.pop(j))
                if 0 <= c - RLAG < NCHUNK:
                    j = c - RLAG
                    chunk_relu(j, trs.pop(j))
                if 0 <= c - OLAG < NCHUNK:
                    chunk_store(c - OLAG)

            for c in range(NCHUNK + OLAG):
                step(c)

        if reps == 1:
            body()
        else:
            with tc.For_i(0, reps, 1) as i:
                body(i)

    nc.compile()
    return nc


_CACHE = {}


def _get_program(reps: int = 1):
    if reps not in _CACHE:
        _CACHE[reps] = _build(reps)
    return _CACHE[reps]


def _limbs(a):
    h = a.astype(BF16).astype(np.float32)
    m = (a - h).astype(BF16).astype(np.float32)
    l = (a - h - m).astype(BF16).astype(np.float32)
    return h, m, l


def prep_core_inputs(xb, yb, conv_w, conv_b):
    """Host-side prep for one batch: limb decomposition + aug tensors."""
    xh, xm, xl = _limbs(xb)
    yh, ym, yl = _limbs(yb)
    ones2 = np.ones((2, N), np.float32)
    la = np.concatenate([xh[: C - 2], ones2, xl[: C - 2], ones2], 0)
    lb = np.concatenate([xm, xh], 0)
    # lab interleaves [la_c | lb_c] per 128-col chunk -> one contiguous DMA
    # covers any chunk range
    lab = np.empty((P, 2 * N), np.float32)
    lab3 = lab.reshape(P, NCHUNK, 2 * P)
    lab3[:, :, :P] = la.reshape(P, NCHUNK, P)
    lab3[:, :, P:] = lb.reshape(P, NCHUNK, P)
    # 4 bf16 limbs of -0.5||y||^2 go in the rows sacrificed from hl/lh
    nrm = -0.5 * (yb.astype(np.float64) ** 2).sum(0)
    rn = np.zeros((4, M), np.float32)
    r = nrm
    for j in range(4):
        rn[j] = r.astype(BF16).astype(np.float32)
        r = r - rn[j].astype(np.float64)
    ra = np.concatenate([yl[: C - 2], rn[0:2], yh[: C - 2], rn[2:4]], 0)
    rb = np.concatenate([ym, ym], 0)
    rc = np.concatenate([yh, yh], 0)
    # rabc packs the two 512-col halves: [RA_h | RB_h | RC_h] for h = 0, 1
    rabc = np.empty((P, 3 * M), np.float32)
    for h in range(2):
        o = h * 3 * 512
        s = slice(h * 512, (h + 1) * 512)
        rabc[:, o : o + 512] = ra[:, s]
        rabc[:, o + 512 : o + 1024] = rb[:, s]
        rabc[:, o + 1024 : o + 1536] = rc[:, s]
    xw = np.zeros((C + 1, M + ZROW), np.float32)
    xw[:C, :M] = xb[:, :M]
    xw[C, :M] = 1.0
    for k in range(K):
        xw[:C, M + k * OC : M + (k + 1) * OC] = conv_w[:, :, k].T
        xw[C, M + k * OC : M + (k + 1) * OC] = conv_b / K
    idy = np.eye(P, dtype=np.float32)
    return {
        "lab": lab.astype(BF16), "rabc": rabc.astype(BF16),
        "xw": xw, "idy": idy,
    }


def _in_maps(x, y, conv_w, conv_b):
    return [prep_core_inputs(x[b], y[b], conv_w, conv_b) for b in range(B)]


def kernel(x, y, conv_w, conv_b):
    x = np.asarray(x, dtype=np.float32)
    y = np.asarray(y, dtype=np.float32)
    conv_w = np.asarray(conv_w, dtype=np.float32)
    conv_b = np.asarray(conv_b, dtype=np.float32)
    nc = _get_program(1)
    maps = _in_maps(x, y, conv_w, conv_b)
    res = run_bass_kernel_spmd(nc, maps, list(range(B)))
    return np.stack([res.results[b]["out"] for b in range(B)], 0)


def run_sim(x, y, conv_w, conv_b, core=0):
    """CoreSim single-core run for debugging."""
    from concourse.bass_interp import CoreSim

    nc = _get_program(1)
    maps = _in_maps(np.asarray(x, np.float32), np.asarray(y, np.float32),
                    np.asarray(conv_w, np.float32), np.asarray(conv_b, np.float32))
    sim = CoreSim(nc)
    for name, arr in maps[core].items():
        sim.tensor(name)[:] = arr
    sim.simulate(check_with_hw=False)
    return np.array(sim.tensor("out"))
